# revision 10
# baseline (speedup 1.0000x reference)
import sys
import numpy as np

sys.path.insert(0, '/opt/trn_rl_repo')

import ml_dtypes  # noqa: E402
import concourse.bass as bass  # noqa: E402
import concourse.mybir as mybir  # noqa: E402
from concourse.bass_utils import run_bass_kernel_spmd  # noqa: E402

V, E, H, L, T, B = 10000, 512, 1024, 2, 128, 64
NCORES = 8
HS = H // NCORES          # 128 features per core
VS = V // NCORES          # 1250 vocab per core
ROWS = T * B              # 8192
KH = H // 128             # 8 contraction chunks over H
KE = E // 128             # 4 contraction chunks over E
STEPS = T + 1             # interleaved steps (L1 lags L0 by one)

BF = mybir.dt.bfloat16
F32 = mybir.dt.float32
I16 = mybir.dt.int16
AG = "AllGather"
BYPASS = mybir.AluOpType.bypass

_CACHE = {}


def _build():
    if "nc" in _CACHE:
        return _CACHE["nc"]
    from contextlib import ExitStack
    es = ExitStack()

    nc = bass.Bass("TRN2", target_bir_lowering=False,
                   detect_race_conditions=False, num_devices=NCORES)

    # ---------------- DRAM I/O ----------------
    din = {}

    def inp(name, shape, dt):
        din[name] = nc.dram_tensor(name, list(shape), dt, kind="ExternalInput")
        return din[name]

    inp("xt_in", [128, KE * ROWS], BF)
    for g in "rzh":
        inp(f"w0{g}T", [128, KE * 128], BF)
        inp(f"u0{g}T", [128, KH * 128], BF)
        inp(f"w1{g}T", [128, KH * 128], BF)
        inp(f"u1{g}T", [128, KH * 128], BF)
        inp(f"b0{g}", [128, 1], F32)
        inp(f"b1{g}", [128, 1], F32)
    inp("woutT", [128, KH * VS], BF)
    inp("ident", [128, 128], BF)
    inp("bout_rep", [128, VS], F32)
    inp("hfull_init", [128, 2 * KH * B], BF)
    inp("hloc_init", [128, 2 * B], BF)

    logits_o = nc.dram_tensor("logits", [ROWS, VS], F32, kind="ExternalOutput")
    hfin_o = nc.dram_tensor("hfinal", [128, 2 * B], F32, kind="ExternalOutput")

    rh_in = nc.dram_tensor("rh_in", [STEPS * 2 * HS, B], BF, kind="Internal")
    rh_out = nc.dram_tensor("rh_out", [STEPS * 2 * H, B], BF, kind="Internal",
                            addr_space="Shared")
    h_in = nc.dram_tensor("h_in", [STEPS * 2 * HS, B], BF, kind="Internal")
    h_out = nc.dram_tensor("h_out", [STEPS * 2 * H, B], BF, kind="Internal",
                           addr_space="Shared")

    # ---------------- SBUF ----------------
    sb = {}

    def sbuf(name, shape, dt):
        sb[name] = es.enter_context(nc.sbuf_tensor("sb_" + name, list(shape), dt))
        return sb[name]

    sbuf("xt", [128, KE, ROWS], BF)
    for g in "rzh":
        sbuf(f"gx0{g}", [128, ROWS], BF)
        for w in ("w0", "u0", "w1", "u1"):
            kk = KE if w == "w0" else KH
            sbuf(f"{w}{g}T", [128, kk * 128], BF)
        sbuf(f"b0{g}", [128, 1], F32)
        sbuf(f"b1{g}", [128, 1], F32)
    sbuf("woutT", [128, KH * VS], BF)
    sbuf("ident", [128, 128], BF)
    sbuf("bout_rep", [128, VS], F32)
    sbuf("hfull0", [128, 2 * KH * B], BF)
    sbuf("hfull1", [128, 2 * KH * B], BF)
    sbuf("rhfull", [128, 2 * KH * B], BF)
    sbuf("hsrc0", [128, 2 * B], BF)
    sbuf("hsrc1", [128, 2 * B], BF)
    sbuf("rhsrc", [128, 2 * B], BF)
    for n in ("r0s", "z0s", "r1s", "z1s", "ht0", "ht1", "tmp0", "tmp1"):
        sbuf(n, [128, B], BF)
    sbuf("hf", [128, 2 * B], F32)
    sbuf("top0", [128, KH, 128], BF)
    sbuf("top1", [128, KH, 128], BF)
    sbuf("lg0", [128, VS], F32)
    sbuf("lg1", [128, VS], F32)

    pb = [es.enter_context(nc.psum_tensor(f"pb{i}", [128, 512], F32))
          for i in range(8)]

    sems = {}
    for s in ("s_ld", "s_pe", "s_act", "s_vec", "s_bo", "s_cc", "s_bi", "s_st"):
        sems[s] = es.enter_context(nc.semaphore(s))

    # op list: (engine, builder(eng)->inst|None, [(sem,val)...], [(sem,inc)...])
    ops = []
    cnt = dict(s_ld=0, s_pe=0, s_act=0, s_vec=0, s_bo=0, s_cc=0, s_bi=0, s_st=0)

    def op(engine, fn, waits=(), incs=(), attach=True):
        ops.append((engine, fn, list(waits), list(incs) if attach else []))
        for s, n in incs:
            cnt[s] += n

    def W(s):
        return (s, cnt[s])

    # ============ P0: input loads ============
    for name, t in din.items():
        if name in ("emb_bf",):
            continue
        dst = sb.get(name)
        if dst is None:
            continue
        op("sync", lambda t=t, dst=dst: nc.sync.dma_start(dst[:, :], t.ap()),
           incs=[("s_ld", 16)])
    n_ld_all = cnt["s_ld"]
    # init hfull1 / hsrc1 from init inputs
    op("sync", lambda: nc.sync.dma_start(sb["hfull1"][:, :], din["hfull_init"].ap()),
       waits=[], incs=[("s_ld", 16)])
    op("sync", lambda: nc.sync.dma_start(sb["hsrc1"][:, :], din["hloc_init"].ap()),
       incs=[("s_ld", 16)])
    n_ld_all = cnt["s_ld"]

    # xt prepared host-side: xt[p, c, i] = emb[idx_i, c*128+p]
    op("sync", lambda: nc.sync.dma_start(
        bass.AP(sb["xt"], 0, [[KE * ROWS, 128], [1, KE * ROWS]]),
        din["xt_in"].ap()), incs=[("s_ld", 16)])
    n_ld_all = cnt["s_ld"]

    # zero rhsrc (slot 1 is shipped before L1 ever writes it); on DVE so
    # FIFO order guarantees it lands before the first bounce-out read.
    op("vector", lambda: nc.vector.memset(sb["rhsrc"][:, :], 0))

    # ============ P1: Gx0 = W0 @ x for own slice ============
    evac_at = {}   # bank -> required s_vec value before reuse
    for gi, g in enumerate("rzh"):
        for n in range(ROWS // 512):
            bank = pb[6 + (n % 2)]
            bw = []
            if (gi, n) == (0, 0):
                bw = [("s_ld", n_ld_all)]
            key = 6 + (n % 2)
            if key in evac_at:
                bw.append(("s_vec", evac_at[key]))

            def mms(g=g, n=n, bank=bank):
                last = None
                for c in range(KE):
                    last = nc.tensor.matmul(
                        bank[:, :],
                        sb[f"w0{g}T"][:, c * 128:(c + 1) * 128],
                        sb["xt"][:, c, n * 512:(n + 1) * 512],
                        start=(c == 0), stop=(c == KE - 1))
                return last
            op("tensor", mms, waits=bw, incs=[("s_pe", 1)])
            pe_v = cnt["s_pe"]
            op("vector",
               lambda g=g, n=n, bank=bank: nc.vector.tensor_copy(
                   sb[f"gx0{g}"][:, n * 512:(n + 1) * 512], bank[:, :]),
               waits=[("s_pe", pe_v)], incs=[("s_vec", 1)])
            evac_at[key] = cnt["s_vec"]

    # ============ P2: recurrence ============
    SIG = mybir.ActivationFunctionType.Sigmoid
    TANH = mybir.ActivationFunctionType.Tanh
    P_R0, P_Z0, P_H0, P_R1, P_Z1, P_H1 = range(6)

    bi_h = {-1: cnt["s_bi"]}
    act_g, act_t, vec_rh, vec_blend = {}, {}, {}, {}
    pe_A, pe_B, bo_rh, bo_h, cc_h = {}, {}, {}, {}, {}
    act_t[-1] = 0
    vec_rh[-1] = 0
    vec_blend[-1] = 0
    bo_rh[-1] = 0
    bo_h[-1] = 0
    bo_h[-2] = 0
    pe_A[-1] = 0
    pe_B[-1] = 0

    def hfull(s):
        return sb["hfull0"] if s % 2 == 0 else sb["hfull1"]

    def hsrc(s):
        return sb["hsrc0"] if s % 2 == 0 else sb["hsrc1"]

    for s in range(STEPS):
        l0 = s < T
        l1 = s >= 1
        hprev = hfull(s - 1)
        hlocp = hsrc(s - 1)

        # ---- phase A: r/z matmuls (+ hh1 x-part) ----
        def phaseA(s=s, l0=l0, l1=l1, hprev=hprev):
            insts = []
            if l0:
                for gate, pbi in (("r", P_R0), ("z", P_Z0)):
                    nc.tensor.matmul(pb[pbi][:, 0:B], sb["ident"][:, :],
                                     sb[f"gx0{gate}"][:, s * B:(s + 1) * B],
                                     start=True, stop=False)
                    for k in range(KH):
                        i = nc.tensor.matmul(
                            pb[pbi][:, 0:B],
                            sb[f"u0{gate}T"][:, k * 128:(k + 1) * 128],
                            hprev[:, (2 * k) * B:(2 * k + 1) * B],
                            start=False, stop=(k == KH - 1))
                    insts.append(i)
            if l1:
                for gate, pbi in (("r", P_R1), ("z", P_Z1)):
                    for k in range(KH):
                        nc.tensor.matmul(
                            pb[pbi][:, 0:B],
                            sb[f"w1{gate}T"][:, k * 128:(k + 1) * 128],
                            hprev[:, (2 * k) * B:(2 * k + 1) * B],
                            start=(k == 0), stop=False)
                    for k in range(KH):
                        i = nc.tensor.matmul(
                            pb[pbi][:, 0:B],
                            sb[f"u1{gate}T"][:, k * 128:(k + 1) * 128],
                            hprev[:, (2 * k + 1) * B:(2 * k + 2) * B],
                            start=False, stop=(k == KH - 1))
                    insts.append(i)
                # hh1 x-part (rest accumulates in phase B)
                for k in range(KH):
                    nc.tensor.matmul(
                        pb[P_H1][:, 0:B],
                        sb["w1hT"][:, k * 128:(k + 1) * 128],
                        hprev[:, (2 * k) * B:(2 * k + 1) * B],
                        start=(k == 0), stop=False)
            return insts[-1] if insts else None

        waA = [("s_bi", bi_h[s - 1]), ("s_act", act_t[s - 1])]
        ngates = (2 if l0 else 0) + (2 if l1 else 0)
        op("tensor", phaseA, waits=waA, incs=[("s_pe", 1)])
        pe_A[s] = cnt["s_pe"]

        # ---- sigmoids ----
        def sigs(s=s, l0=l0, l1=l1):
            last = None
            if l0:
                last = nc.scalar.activation(sb["r0s"][:, :], pb[P_R0][:, 0:B],
                                            SIG, bias=sb["b0r"][:, 0:1])
                last = nc.scalar.activation(sb["z0s"][:, :], pb[P_Z0][:, 0:B],
                                            SIG, bias=sb["b0z"][:, 0:1])
            if l1:
                last = nc.scalar.activation(sb["r1s"][:, :], pb[P_R1][:, 0:B],
                                            SIG, bias=sb["b1r"][:, 0:1])
                last = nc.scalar.activation(sb["z1s"][:, :], pb[P_Z1][:, 0:B],
                                            SIG, bias=sb["b1z"][:, 0:1])
            return last
        op("scalar", sigs,
           waits=[("s_pe", pe_A[s]), ("s_vec", vec_blend[s - 1])],
           incs=[("s_act", 1)])
        act_g[s] = cnt["s_act"]

        # ---- rh = r * h_loc ----
        def rhmul(s=s, l0=l0, l1=l1, hlocp=hlocp):
            last = None
            if l0:
                last = nc.vector.tensor_mul(sb["rhsrc"][:, 0:B],
                                            sb["r0s"][:, :], hlocp[:, 0:B])
            if l1:
                last = nc.vector.tensor_mul(sb["rhsrc"][:, B:2 * B],
                                            sb["r1s"][:, :], hlocp[:, B:2 * B])
            return last
        op("vector", rhmul,
           waits=[("s_act", act_g[s]), ("s_bo", bo_rh[s - 1])],
           incs=[("s_vec", 1)])
        vec_rh[s] = cnt["s_vec"]

        # ---- bounce out rh + AllGather + bounce in ----
        op("sync",
           lambda s=s: nc.sync.dma_start(
               bass.AP(rh_in, s * 2 * HS * B,
                       [[B, 128], [HS * B, 2], [1, B]]),
               bass.AP(sb["rhsrc"], 0, [[2 * B, 128], [B, 2], [1, B]])),
           waits=[("s_vec", vec_rh[s])], incs=[("s_bo", 16)])
        bo_rh[s] = cnt["s_bo"]

        op("gpsimd",
           lambda s=s: nc.gpsimd.collective_compute(
               AG, BYPASS, replica_groups=[list(range(NCORES))],
               ins=[bass.AP(rh_in, s * 2 * HS * B, [[B, 2 * HS], [1, B]])],
               outs=[bass.AP(rh_out, s * 2 * H * B, [[B, 2 * H], [1, B]])],
           ).then_inc(sems["s_cc"], 1),
           waits=[("s_bo", bo_rh[s])])
        cnt["s_cc"] += 1
        cc_rh_v = cnt["s_cc"]

        op("sync",
           lambda s=s: nc.sync.dma_start(
               sb["rhfull"][:, :],
               bass.AP(rh_out, s * 2 * H * B,
                       [[B, 128], [HS * B, 2 * KH], [1, B]])),
           waits=[("s_cc", cc_rh_v), ("s_pe", pe_B[s - 1])],
           incs=[("s_bi", 16)])
        bi_rh_v = cnt["s_bi"]

        # ---- phase B: hh matmuls ----
        def phaseB(s=s, l0=l0, l1=l1):
            last = None
            if l0:
                nc.tensor.matmul(pb[P_H0][:, 0:B], sb["ident"][:, :],
                                 sb["gx0h"][:, s * B:(s + 1) * B],
                                 start=True, stop=False)
                for k in range(KH):
                    last = nc.tensor.matmul(
                        pb[P_H0][:, 0:B],
                        sb["u0hT"][:, k * 128:(k + 1) * 128],
                        sb["rhfull"][:, (2 * k) * B:(2 * k + 1) * B],
                        start=False, stop=(k == KH - 1))
            if l1:
                for k in range(KH):
                    last = nc.tensor.matmul(
                        pb[P_H1][:, 0:B],
                        sb["u1hT"][:, k * 128:(k + 1) * 128],
                        sb["rhfull"][:, (2 * k + 1) * B:(2 * k + 2) * B],
                        start=False, stop=(k == KH - 1))
            return last
        op("tensor", phaseB, waits=[("s_bi", bi_rh_v)], incs=[("s_pe", 1)])
        pe_B[s] = cnt["s_pe"]

        # ---- tanh ----
        def tanhs(s=s, l0=l0, l1=l1):
            last = None
            if l0:
                last = nc.scalar.activation(sb["ht0"][:, :], pb[P_H0][:, 0:B],
                                            TANH, bias=sb["b0h"][:, 0:1])
            if l1:
                last = nc.scalar.activation(sb["ht1"][:, :], pb[P_H1][:, 0:B],
                                            TANH, bias=sb["b1h"][:, 0:1])
            return last
        op("scalar", tanhs,
           waits=[("s_pe", pe_B[s]), ("s_vec", vec_blend[s - 1])],
           incs=[("s_act", 1)])
        act_t[s] = cnt["s_act"]

        # ---- blend: h_new = h + z*(ht - h) ----
        def blends(s=s, l0=l0, l1=l1, hlocp=hlocp):
            hn = hsrc(s)
            last = None
            if l0:
                nc.vector.tensor_sub(sb["tmp0"][:, :], sb["ht0"][:, :], hlocp[:, 0:B])
                nc.vector.tensor_mul(sb["tmp0"][:, :], sb["tmp0"][:, :], sb["z0s"][:, :])
                last = nc.vector.tensor_add(hn[:, 0:B], hlocp[:, 0:B], sb["tmp0"][:, :])
            if l1:
                nc.vector.tensor_sub(sb["tmp1"][:, :], sb["ht1"][:, :], hlocp[:, B:2 * B])
                nc.vector.tensor_mul(sb["tmp1"][:, :], sb["tmp1"][:, :], sb["z1s"][:, :])
                last = nc.vector.tensor_add(hn[:, B:2 * B], hlocp[:, B:2 * B], sb["tmp1"][:, :])
            else:
                # s == 0: carry h1 init forward so AG(0) ships real h1(-1)
                last = nc.vector.tensor_copy(hn[:, B:2 * B], hlocp[:, B:2 * B])
            return last
        op("vector", blends,
           waits=[("s_act", act_t[s]), ("s_bo", bo_h[s - 2])],
           incs=[("s_vec", 1)])
        vec_blend[s] = cnt["s_vec"]

        # ---- bounce out h + AllGather + bounce in ----
        op("sync",
           lambda s=s: nc.sync.dma_start(
               bass.AP(h_in, s * 2 * HS * B, [[B, 128], [HS * B, 2], [1, B]]),
               bass.AP(hsrc(s), 0, [[2 * B, 128], [B, 2], [1, B]])),
           waits=[("s_vec", vec_blend[s])], incs=[("s_bo", 16)])
        bo_h[s] = cnt["s_bo"]

        op("gpsimd",
           lambda s=s: nc.gpsimd.collective_compute(
               AG, BYPASS, replica_groups=[list(range(NCORES))],
               ins=[bass.AP(h_in, s * 2 * HS * B, [[B, 2 * HS], [1, B]])],
               outs=[bass.AP(h_out, s * 2 * H * B, [[B, 2 * H], [1, B]])],
           ).then_inc(sems["s_cc"], 1),
           waits=[("s_bo", bo_h[s])])
        cnt["s_cc"] += 1
        cc_h[s] = cnt["s_cc"]

        op("sync",
           lambda s=s: nc.sync.dma_start(
               hfull(s)[:, :],
               bass.AP(h_out, s * 2 * H * B,
                       [[B, 128], [HS * B, 2 * KH], [1, B]])),
           waits=[("s_cc", cc_h[s]), ("s_pe", pe_A[s - 1] if s >= 1 else 0)],
           incs=[("s_bi", 16)])
        bi_h[s] = cnt["s_bi"]

    # ---- h_final out ----
    def hfcopy():
        nc.vector.tensor_copy(sb["hf"][:, 0:B], sb["hsrc1"][:, 0:B])
        return nc.vector.tensor_copy(sb["hf"][:, B:2 * B], sb["hsrc0"][:, B:2 * B])
    op("vector", hfcopy, waits=[("s_vec", vec_blend[STEPS - 1])],
       incs=[("s_vec", 1)])
    hf_v = cnt["s_vec"]
    op("sync", lambda: nc.sync.dma_start(hfin_o.ap(), sb["hf"][:, :]),
       waits=[("s_vec", hf_v)], incs=[("s_st", 16)])

    # ============ P3: logits ============
    NB = ((0, 512), (512, 512), (1024, VS - 1024))
    pe_P3, vec_P3, st_P3, bi_P3 = {}, {}, {}, {}
    vec_P3[-1] = vec_P3[-2] = cnt["s_vec"]
    st_P3[-1] = st_P3[-2] = cnt["s_st"]
    pe_first = True
    for R in range(ROWS // 128):
        par = R % 2
        top = sb["top0"] if par == 0 else sb["top1"]
        lg = sb["lg0"] if par == 0 else sb["lg1"]

        def topload(R=R, top=top):
            last = None
            for k in range(KH):
                last = nc.sync.dma_start(
                    bass.AP(top, k * 128, [[KH * 128, 128], [B, 2], [1, B]]),
                    bass.AP(h_out,
                            (2 * R + 1) * 2 * H * B + (k * 2 * HS + HS) * B,
                            [[B, 128], [2 * H * B, 2], [1, B]])
                ).then_inc(sems["s_bi"], 16)
            return last
        # wait: AGs done for steps 2R+1, 2R+2; PE done with this buffer (R-2)
        op("sync", topload,
           waits=[("s_cc", cc_h[min(2 * R + 2, STEPS - 1)]),
                  ("s_pe", pe_P3.get(R - 2, 0))],
           incs=[("s_bi", 16 * KH)], attach=False)
        bi_P3[R] = cnt["s_bi"]

        def p3mm(R=R, par=par, top=top):
            last = None
            for k in range(KH):
                for nb, (off, wdt) in enumerate(NB):
                    last = nc.tensor.matmul(
                        pb[par * 3 + nb][:, 0:wdt],
                        top[:, k, :],
                        sb["woutT"][:, k * VS + off: k * VS + off + wdt],
                        start=(k == 0), stop=(k == KH - 1))
            return last
        wp = [("s_bi", bi_P3[R]), ("s_vec", vec_P3[R - 2])]
        if pe_first:
            wp += [("s_act", act_t[STEPS - 1]), ("s_vec", vec_blend[STEPS - 1])]
            pe_first = False
        op("tensor", p3mm, waits=wp, incs=[("s_pe", 1)])
        pe_P3[R] = cnt["s_pe"]

        def p3evac(R=R, par=par, lg=lg):
            last = None
            for nb, (off, wdt) in enumerate(NB):
                last = nc.vector.tensor_add(
                    lg[:, off:off + wdt], pb[par * 3 + nb][:, 0:wdt],
                    sb["bout_rep"][:, off:off + wdt])
            return last
        op("vector", p3evac,
           waits=[("s_pe", pe_P3[R]), ("s_st", st_P3[R - 2])],
           incs=[("s_vec", 1)])
        vec_P3[R] = cnt["s_vec"]

        op("sync",
           lambda R=R, lg=lg: nc.sync.dma_start(
               bass.AP(logits_o, R * 128 * VS, [[VS, 128], [1, VS]]),
               lg[:, :]),
           waits=[("s_vec", vec_P3[R])], incs=[("s_st", 16)])
        st_P3[R] = cnt["s_st"]

    # ============ emit per engine ============
    engines = {"tensor": nc.tensor, "scalar": nc.scalar, "vector": nc.vector,
               "sync": nc.sync, "gpsimd": nc.gpsimd}
    with nc.Block() as block:
        def run_engine(name):
            eng = engines[name]
            for engine, fn, waits, incs in ops:
                if engine != name:
                    continue
                for sname, val in waits:
                    if val > 0:
                        eng.wait_ge(sems[sname], val)
                inst = fn()
                for sname, n in incs:
                    assert inst is not None, f"inc on None inst ({name})"
                    inst.then_inc(sems[sname], n)

        block.tensor(lambda eng: run_engine("tensor"))
        block.scalar(lambda eng: run_engine("scalar"))
        block.vector(lambda eng: run_engine("vector"))
        block.sync(lambda eng: run_engine("sync"))
        block.gpsimd(lambda eng: run_engine("gpsimd"))

    es.close()
    _CACHE["nc"] = nc
    return nc


def _chunkT(Ws):
    """[out_m<=128, K] weight slice -> SBUF lhsT layout [128, K] bf16 with
    arr[p, k*128+m] = Ws[m, k*128+p]."""
    K = Ws.shape[1]
    kk = K // 128
    WT = Ws.T.reshape(kk, 128, Ws.shape[0]).transpose(1, 0, 2).reshape(128, kk * Ws.shape[0])
    return np.ascontiguousarray(WT).astype(ml_dtypes.bfloat16)


def kernel(**inputs):
    nc = _build()

    idx = np.asarray(inputs["inputs"]).astype(np.int64).reshape(ROWS)
    hidden = np.asarray(inputs["hidden"], np.float32)
    emb = np.asarray(inputs["emb"], np.float32)
    Wout = np.asarray(inputs["Wout"], np.float32)
    bout = np.asarray(inputs["bout"], np.float32)

    emb_bf = emb.astype(ml_dtypes.bfloat16)
    xt_np = np.ascontiguousarray(
        emb_bf[idx].astype(np.float32).T.reshape(KE, 128, ROWS)
        .transpose(1, 0, 2).reshape(128, KE * ROWS)).astype(ml_dtypes.bfloat16)
    hT = hidden.transpose(0, 2, 1)  # [2, H, B]
    hfull_init = np.ascontiguousarray(
        hT.reshape(2, KH, 128, B).transpose(2, 1, 0, 3).reshape(128, 2 * KH * B)
    ).astype(ml_dtypes.bfloat16)
    ident = np.eye(128, dtype=np.float32).astype(ml_dtypes.bfloat16)

    in_maps = []
    for c in range(NCORES):
        hs = slice(c * HS, (c + 1) * HS)
        vs = slice(c * VS, (c + 1) * VS)
        m = {
            "xt_in": xt_np, "ident": ident,
            "hfull_init": hfull_init,
            "hloc_init": np.ascontiguousarray(
                np.concatenate([hT[0, hs, :], hT[1, hs, :]], axis=1)
            ).astype(ml_dtypes.bfloat16),
            "bout_rep": np.ascontiguousarray(
                np.broadcast_to(bout[vs][None, :], (128, VS))).astype(np.float32),
        }
        Wos = Wout[vs, :]  # [VS, H]
        woutT = Wos.T.reshape(KH, 128, VS).transpose(1, 0, 2).reshape(128, KH * VS)
        m["woutT"] = np.ascontiguousarray(woutT).astype(ml_dtypes.bfloat16)
        for g in "rzh":
            m[f"w0{g}T"] = _chunkT(np.asarray(inputs[f"W{g}0"], np.float32)[hs, :])
            m[f"u0{g}T"] = _chunkT(np.asarray(inputs[f"U{g}0"], np.float32)[hs, :])
            m[f"w1{g}T"] = _chunkT(np.asarray(inputs[f"W{g}1"], np.float32)[hs, :])
            m[f"u1{g}T"] = _chunkT(np.asarray(inputs[f"U{g}1"], np.float32)[hs, :])
            m[f"b0{g}"] = np.ascontiguousarray(
                np.asarray(inputs[f"b{g}0"], np.float32)[hs][:, None])
            m[f"b1{g}"] = np.ascontiguousarray(
                np.asarray(inputs[f"b{g}1"], np.float32)[hs][:, None])
        in_maps.append(m)

    res = run_bass_kernel_spmd(nc, in_maps, core_ids=list(range(NCORES)))

    logits = np.concatenate([res.results[c]["logits"] for c in range(NCORES)],
                            axis=1).reshape(T, B, V).astype(np.float32)
    h_final = np.zeros((L, B, H), np.float32)
    for c in range(NCORES):
        hf = res.results[c]["hfinal"]
        h_final[0, :, c * HS:(c + 1) * HS] = hf[:, 0:B].T
        h_final[1, :, c * HS:(c + 1) * HS] = hf[:, B:2 * B].T
    return logits, h_final
